# revision 20
# baseline (speedup 1.0000x reference)
"""Trainium2 Bass kernel for nn_Decay2DBlk (block-decay linear attention).

Full-input contract: kernel(**inputs) takes the unsharded inputs from
setup_inputs() and returns the full [B, T, E] output.

Sharding: 8 cores = 4 batch elements x 2 Dv-halves. Each core computes a
partial output y_b_h = (attn(x_b) restricted to its Dv half) @ Wo[half];
the host sums the two partials per batch element (the "all-reduce after
w_out" done host-side since outputs are gathered anyway).

Math (per core): with d=0.99, D=d^128, block index n, in-block offsets
s,t (keys/queries):
  out[t] = sum_{s<=t} d^(t_g - s_g) * q_t k_s * v_s  @ Wo      (t_g global)
All decay factors are folded into host-precomputed constants so the device
only does matmuls + one elementwise mask multiply + a running-sum state:
  - masks[n][s,t]  = 1[s<=t] d^(-s-1) D^-(n-1)        (intra-block, bf16)
  - vscale[n][t]   = (1-d) d^(127-t) D^-n             (v' for state update)
  - escale[n][t]   = d^(t+1) D^(n-1)                  (final ACT evac scale)
  - state S~ = sum_m k_m^T v'_m  (pure running sum, bf16 in SBUF)
The geometric growth of D^-n keeps every intermediate within fp32/bf16
range (max ~1e19) and makes bf16 rounding of the running sum benign
(validated: rel err ~4e-3 vs fp32 reference).
"""

import os
import sys

for _p in (
    "/root/.axon_site",
    "/root/.axon_site/_ro/trn_rl_repo",
    "/root/.axon_site/_ro/pypackages",
    "/opt/trn_rl_repo",
):
    if os.path.isdir(_p) and _p not in sys.path:
        sys.path.append(_p)

import numpy as np
import ml_dtypes
from contextlib import ExitStack

import concourse.bacc as bacc
import concourse.tile as tile
from concourse import mybir
from concourse.bass_utils import run_bass_kernel_spmd

DECAY = 0.99
TBLK = 128
BF16 = ml_dtypes.bfloat16
BF = mybir.dt.bfloat16
F32 = mybir.dt.float32


def build_nc(T=4096, E=1024, Dk=1024, Dvh=512, ST=512, pair_groups=None):
    """Build the per-core Bass program. Same program runs on all 8 cores
    (SPMD); only the input data differs.

    q/k phase-A work is split across the two cores of a pair: each core
    computes sigmoid(x @ Wq_half + b_half) for the dk-half whose weights it
    was GIVEN as input (wq input is [E, Dk/2]), then the halves are
    exchanged with a pairwise AllGather through DRAM bounce buffers. Core
    2b (group rank 0) always carries the low half, so the gathered layout
    is identical on both cores and the program stays SPMD-symmetric."""
    N = T // TBLK       # number of 128-token blocks
    NU = T // ST        # number of super-tiles
    JB = ST // TBLK     # blocks per super-tile
    EC = E // 128       # E chunks (contraction)
    KC = Dk // 128      # Dk chunks
    HC = KC // 2        # dk chunks computed locally (half)
    DC = Dvh // 128     # Dv-half chunks
    Dkh = Dk // 2
    if pair_groups is None:
        pair_groups = [[0, 1], [2, 3], [4, 5], [6, 7]]

    nc = bacc.Bacc(num_devices=8)
    xt = nc.dram_tensor("xt", [E, T], BF, kind="ExternalInput")
    wq = nc.dram_tensor("wq", [E, Dkh], BF, kind="ExternalInput")
    wk = nc.dram_tensor("wk", [E, Dkh], BF, kind="ExternalInput")
    wv = nc.dram_tensor("wv", [E, Dvh], BF, kind="ExternalInput")
    wo = nc.dram_tensor("wo", [Dvh, E], BF, kind="ExternalInput")
    bq = nc.dram_tensor("bq", [Dkh, 1], F32, kind="ExternalInput")
    bk = nc.dram_tensor("bk", [Dkh, 1], F32, kind="ExternalInput")
    masks = nc.dram_tensor("masks", [128, N, 128], BF, kind="ExternalInput")
    vscale = nc.dram_tensor("vscale", [128, N], F32, kind="ExternalInput")
    escale = nc.dram_tensor("escale", [128, N], F32, kind="ExternalInput")
    ident = nc.dram_tensor("ident", [128, 128], BF, kind="ExternalInput")
    out = nc.dram_tensor("out", [T, E], F32, kind="ExternalOutput")

    SIG = mybir.ActivationFunctionType.Sigmoid
    CPY = mybir.ActivationFunctionType.Copy

    with tile.TileContext(nc) as tc:
        with ExitStack() as ctx:
            consts = ctx.enter_context(tc.tile_pool(name="consts", bufs=1))
            qkh_pool = ctx.enter_context(tc.tile_pool(name="qkh", bufs=2))
            dram = ctx.enter_context(tc.tile_pool(name="dram", bufs=2, space="DRAM"))
            xt_pool = ctx.enter_context(tc.tile_pool(name="xt", bufs=2))
            qk_pool = ctx.enter_context(tc.tile_pool(name="qk", bufs=2))
            v_pool = ctx.enter_context(tc.tile_pool(name="v", bufs=2))
            kn_pool = ctx.enter_context(tc.tile_pool(name="kn", bufs=3))
            ap_pool = ctx.enter_context(tc.tile_pool(name="apool", bufs=3))
            yt_pool = ctx.enter_context(tc.tile_pool(name="yt", bufs=3))
            o_pool = ctx.enter_context(tc.tile_pool(name="opool", bufs=2))
            st_pool = ctx.enter_context(tc.tile_pool(name="state", bufs=1))
            psA = ctx.enter_context(tc.tile_pool(name="psA", bufs=2, space="PSUM"))
            psB = ctx.enter_context(tc.tile_pool(name="psB", bufs=3, space="PSUM"))
            psY = ctx.enter_context(tc.tile_pool(name="psY", bufs=2, space="PSUM"))
            psa = ctx.enter_context(tc.tile_pool(name="psa", bufs=1, space="PSUM"))

            # ---- constants into SBUF ----
            # per-chunk DMAs so the first phase-A matmuls start as soon as
            # their e-chunk lands instead of waiting for the whole weight
            wq_sb = consts.tile([128, EC, Dkh], BF)
            wq_r = wq.rearrange("(ec p) d -> p ec d", p=128)
            for e in range(EC):
                nc.sync.dma_start(out=wq_sb[:, e, :], in_=wq_r[:, e, :])
            wk_sb = consts.tile([128, EC, Dkh], BF)
            wk_r = wk.rearrange("(ec p) d -> p ec d", p=128)
            for e in range(EC):
                nc.sync.dma_start(out=wk_sb[:, e, :], in_=wk_r[:, e, :])
            bq_sb = consts.tile([128, HC, 1], F32)
            nc.sync.dma_start(out=bq_sb, in_=bq.rearrange("(kc p) o -> p kc o", p=128))
            bk_sb = consts.tile([128, HC, 1], F32)
            nc.sync.dma_start(out=bk_sb, in_=bk.rearrange("(kc p) o -> p kc o", p=128))
            vs_sb = consts.tile([128, N], F32)
            nc.sync.dma_start(out=vs_sb, in_=vscale[:, :])
            es_sb = consts.tile([128, N], F32)
            nc.sync.dma_start(out=es_sb, in_=escale[:, :])
            id_sb = consts.tile([128, 128], BF)
            nc.sync.dma_start(out=id_sb, in_=ident[:, :])
            wv_sb = consts.tile([128, EC, Dvh], BF)
            nc.sync.dma_start(out=wv_sb, in_=wv.rearrange("(ec p) d -> p ec d", p=128))
            mk_sb = consts.tile([128, N, 128], BF)
            nc.sync.dma_start(out=mk_sb, in_=masks[:, :, :])
            wo_sb = consts.tile([128, DC, E], BF)
            nc.sync.dma_start(out=wo_sb, in_=wo.rearrange("(dc p) e -> p dc e", p=128))

            # persistent scaled-sum state S~ [dk, dv], one tile per dk-chunk
            # (separate tiles -> per-chunk dependency chains, so the y2 reads
            # of block n+1 pipeline against the state adds of block n)
            S_c = [st_pool.tile([128, Dvh], BF, name=f"S{c}", tag=f"S{c}")
                   for c in range(KC)]

            xt_r = xt.rearrange("(ec p) t -> p ec t", p=128)

            def emit_phase_a(u):
                # ---- load xT super-tile ----
                xt_u = xt_pool.tile([128, EC, ST], BF, name="xt_u")
                for e in range(EC):
                    nc.sync.dma_start(
                        out=xt_u[:, e, :], in_=xt_r[:, e, u * ST:(u + 1) * ST])

                # ---- phase A: this core computes its dk-half of qT, kT ----
                qTh_u = qkh_pool.tile([128, HC, ST], BF, name="qTh_u", tag="qTh")
                kTh_u = qkh_pool.tile([128, HC, ST], BF, name="kTh_u", tag="kTh")
                for c in range(HC):
                    ps = psA.tile([128, ST], F32, name="psq", tag="psA")
                    for e in range(EC):
                        nc.tensor.matmul(
                            ps, wq_sb[:, e, c * 128:(c + 1) * 128], xt_u[:, e, :],
                            start=(e == 0), stop=(e == EC - 1))
                    nc.scalar.activation(qTh_u[:, c, :], ps, SIG, bias=bq_sb[:, c, :])
                for c in range(HC):
                    ps = psA.tile([128, ST], F32, name="psk", tag="psA")
                    for e in range(EC):
                        nc.tensor.matmul(
                            ps, wk_sb[:, e, c * 128:(c + 1) * 128], xt_u[:, e, :],
                            start=(e == 0), stop=(e == EC - 1))
                    nc.scalar.activation(kTh_u[:, c, :], ps, SIG, bias=bk_sb[:, c, :])

                # ---- exchange halves with the pair partner (AllGather) ----
                bin_u = dram.tile([2, 128, HC, ST], BF, name="bin_u", tag="bin")
                nc.sync.dma_start(out=bin_u[0], in_=qTh_u)
                nc.sync.dma_start(out=bin_u[1], in_=kTh_u)
                bout_u = dram.tile([2, 2, 128, HC, ST], BF, name="bout_u", tag="bout")
                nc.gpsimd.collective_compute(
                    "AllGather", mybir.AluOpType.bypass,
                    replica_groups=pair_groups,
                    ins=[bin_u.opt()], outs=[bout_u.opt()])
                qT_u = qk_pool.tile([128, KC, ST], BF, name="qT_u", tag="qT")
                kT_u = qk_pool.tile([128, KC, ST], BF, name="kT_u", tag="kT")
                for g in range(2):
                    nc.sync.dma_start(
                        out=qT_u[:, g * HC:(g + 1) * HC, :], in_=bout_u[g, 0])
                    nc.sync.dma_start(
                        out=kT_u[:, g * HC:(g + 1) * HC, :], in_=bout_u[g, 1])

                v_u = v_pool.tile([128, JB, Dvh], BF, name="v_u", tag="v")
                vp_u = v_pool.tile([128, JB, Dvh], BF, name="vp_u", tag="vp")
                for j in range(JB):
                    n = u * JB + j
                    ps = psA.tile([128, ST], F32, name="psv", tag="psA")[:, :Dvh]
                    for e in range(EC):
                        nc.tensor.matmul(
                            ps, xt_u[:, e, j * 128:(j + 1) * 128], wv_sb[:, e, :],
                            start=(e == 0), stop=(e == EC - 1))
                    nc.scalar.activation(v_u[:, j, :], ps, CPY, scale=1.0 - DECAY)
                    nc.scalar.activation(vp_u[:, j, :], ps, CPY, scale=vs_sb[:, n:n + 1])
                return qT_u, kT_u, v_u, vp_u

            def emit_blocks(u, tiles):
                qT_u, kT_u, v_u, vp_u = tiles
                # ---- block loop ----
                for j in range(JB):
                    n = u * JB + j
                    jsl = slice(j * 128, (j + 1) * 128)

                    # intra-block attention logits a^T[s,t], masked
                    a_ps = psa.tile([128, 128], F32, name="a_ps")
                    for c in range(KC):
                        nc.tensor.matmul(
                            a_ps, kT_u[:, c, jsl], qT_u[:, c, jsl],
                            start=(c == 0), stop=(c == KC - 1))
                    a_sb = ap_pool.tile([128, 128], BF, name="a_sb")
                    nc.vector.tensor_mul(a_sb, a_ps, mk_sb[:, n, :])

                    # k natural [s, dk] via PE transposes of kT
                    kn = kn_pool.tile([128, Dk], BF, name="kn")
                    for h in range(KC // 4):
                        tp = psB.tile([128, 1024], BF, name="tp", tag="psB")[:, :512]
                        for q4 in range(4):
                            c = h * 4 + q4
                            nc.tensor.transpose(
                                tp[:, q4 * 128:(q4 + 1) * 128],
                                kT_u[:, c, jsl], id_sb)
                        if h == 0:
                            nc.scalar.copy(kn[:, h * 512:(h + 1) * 512], tp)
                        else:
                            nc.vector.tensor_copy(kn[:, h * 512:(h + 1) * 512], tp)

                    # yT[dv, t] = v^T a' + S~^T-contract (cross), unscaled
                    y_ps = psY.tile([128, 4 * 128], F32, name="y_ps")
                    for dc in range(DC):
                        osl = slice(dc * 128, (dc + 1) * 128)
                        dvsl = slice(dc * 128, (dc + 1) * 128)
                        nc.tensor.matmul(
                            y_ps[:, osl], v_u[:, j, dvsl], a_sb,
                            start=True, stop=(n == 0))
                        if n > 0:
                            for c in range(KC):
                                nc.tensor.matmul(
                                    y_ps[:, osl], S_c[c][:, dvsl], qT_u[:, c, jsl],
                                    start=False, stop=(c == KC - 1))
                    yT_sb = yt_pool.tile([128, 4 * 128], BF, name="yT_sb")
                    nc.vector.tensor_copy(yT_sb, y_ps)

                    # state update: S~ += k^T v'  (running sum; copy at n=0).
                    # Emitted before the out projection so the DVE adds overlap
                    # the out matmuls and next block's aT/transposes on PE.
                    for c in range(KC):
                        kv_ps = psB.tile([128, 512], F32, name="kv_ps", tag="psB")[:, :Dvh]
                        nc.tensor.matmul(
                            kv_ps, kn[:, c * 128:(c + 1) * 128], vp_u[:, j, :],
                            start=True, stop=True)
                        if n == 0:
                            nc.vector.tensor_copy(S_c[c], kv_ps)
                        else:
                            nc.vector.tensor_add(S_c[c], S_c[c], kv_ps)

                    # out[t, e] = yT^T @ Wo, evacuated with escale[n][t]
                    o_sb = o_pool.tile([128, E], F32, name="o_sb")
                    for hh in range(E // 512):
                        o_ps = psB.tile([128, 512], F32, name="o_ps", tag="psB")
                        for dc in range(DC):
                            nc.tensor.matmul(
                                o_ps, yT_sb[:, dc * 128:(dc + 1) * 128],
                                wo_sb[:, dc, hh * 512:(hh + 1) * 512],
                                start=(dc == 0), stop=(dc == DC - 1))
                        nc.scalar.activation(
                            o_sb[:, hh * 512:(hh + 1) * 512], o_ps, CPY,
                            scale=es_sb[:, n:n + 1])
                    nc.scalar.dma_start(
                        out=out[n * 128:(n + 1) * 128, :], in_=o_sb)

            # Software pipeline: emit phase A one super-tile ahead so the
            # AllGather for u+1 is in flight during the block loop of u.
            pend = emit_phase_a(0)
            for u in range(NU):
                nxt = emit_phase_a(u + 1) if u + 1 < NU else None
                emit_blocks(u, pend)
                pend = nxt
    return nc


def make_host_constants(T=4096, dtype_np=np.float32):
    """Host-precomputed decay constants (see module docstring)."""
    N = T // TBLK
    d = np.float64(DECAY)
    D128 = d ** TBLK
    s = np.arange(TBLK, dtype=np.float64)
    t = np.arange(TBLK, dtype=np.float64)
    nn = np.arange(N, dtype=np.float64)

    # masks[s, n, t] = 1[s<=t] * d^(-s-1) * D128^-(n-1)
    tri = (s[:, None] <= t[None, :]).astype(np.float64)  # [s, t]
    m = tri[:, None, :] * (d ** (-s - 1.0))[:, None, None] \
        * (D128 ** (-(nn - 1.0)))[None, :, None]
    masks = m.astype(BF16)

    # vscale[t, n] = (1-d) d^(127-t) D128^-n
    vsc = ((1.0 - d) * d ** (127.0 - t))[:, None] * (D128 ** (-nn))[None, :]
    vscale = vsc.astype(np.float32)

    # escale[t, n] = d^(t+1) D128^(n-1)
    esc = (d ** (t + 1.0))[:, None] * (D128 ** (nn - 1.0))[None, :]
    escale = esc.astype(np.float32)

    ident = np.eye(128, dtype=BF16)
    return masks, vscale, escale, ident


_NC_CACHE = {}


def _get_nc(T, E, Dk, Dvh):
    key = (T, E, Dk, Dvh)
    if key not in _NC_CACHE:
        nc = build_nc(T=T, E=E, Dk=Dk, Dvh=Dvh)
        nc.finalize()
        _NC_CACHE[key] = nc
    return _NC_CACHE[key]


def kernel(x, Wv, Wk, bk, Wq, bq, Wo):
    y, _ = run(x, Wv, Wk, bk, Wq, bq, Wo)
    return y


def _install_ntff_hook():
    """The agent image's antenv lacks axon_hooks; recreate it from
    trn_boot's ctypes NTFF driver so trace=True produces profiles."""
    try:
        from antenv.axon_hooks import get_axon_ntff_profile_hook  # noqa: F401
        return
    except ImportError:
        pass
    try:
        import types
        import antenv
        from trn_agent_boot.trn_boot import _ntff_profile_via_ctypes
        hook = _ntff_profile_via_ctypes("/opt/axon/libaxon_pjrt.so")
        mod = types.ModuleType("antenv.axon_hooks")
        _h = {"hook": hook}
        mod.get_axon_ntff_profile_hook = lambda: _h["hook"]
        mod.set_axon_ntff_profile_hook = lambda h: _h.update(hook=h)
        sys.modules["antenv.axon_hooks"] = mod
        antenv.axon_hooks = mod
    except Exception as e:  # profiling is best-effort
        print(f"ntff hook install failed: {e}")


def run(x, Wv, Wk, bk, Wq, bq, Wo, trace=False):
    x = np.asarray(x)
    B, T, E = x.shape
    Dk = np.asarray(Wk).shape[1]
    Dv = np.asarray(Wv).shape[1]
    Dvh = Dv // 2
    assert B == 4, "sharding is hardcoded for B=4 x 2 Dv-halves"

    nc = _get_nc(T, E, Dk, Dvh)
    masks, vscale, escale, ident = make_host_constants(T=T)

    wq_bf = np.asarray(Wq, BF16)
    wk_bf = np.asarray(Wk, BF16)
    bq32 = np.asarray(bq, np.float32).reshape(Dk, 1)
    bk32 = np.asarray(bk, np.float32).reshape(Dk, 1)
    Dkh = Dk // 2

    in_maps = []
    for c in range(8):
        b, h = divmod(c, 2)
        dvs = slice(h * Dvh, (h + 1) * Dvh)
        # this core computes the q/k dk-half matching its pair rank
        dks = slice(h * Dkh, (h + 1) * Dkh)
        in_maps.append({
            "xt": np.ascontiguousarray(x[b].T).astype(BF16),
            "wq": np.ascontiguousarray(wq_bf[:, dks]),
            "wk": np.ascontiguousarray(wk_bf[:, dks]),
            "wv": np.asarray(Wv[:, dvs], BF16),
            "wo": np.asarray(Wo[dvs], BF16),
            "bq": np.ascontiguousarray(bq32[dks]),
            "bk": np.ascontiguousarray(bk32[dks]),
            "masks": masks,
            "vscale": vscale,
            "escale": escale,
            "ident": ident,
        })

    if trace:
        _install_ntff_hook()
    res = run_bass_kernel_spmd(nc, in_maps, core_ids=list(range(8)), trace=trace)
    y = np.zeros((B, T, E), np.float32)
    for c in range(8):
        b = c // 2
        y[b] += res.results[c]["out"]
    return y, res


# revision 21
# speedup vs baseline: 1.0810x; 1.0810x over previous
"""Trainium2 Bass kernel for nn_Decay2DBlk (block-decay linear attention).

Full-input contract: kernel(**inputs) takes the unsharded inputs from
setup_inputs() and returns the full [B, T, E] output.

Sharding: 8 cores = 4 batch elements x 2 Dv-halves. Each core computes a
partial output y_b_h = (attn(x_b) restricted to its Dv half) @ Wo[half];
the host sums the two partials per batch element (the "all-reduce after
w_out" done host-side since outputs are gathered anyway).

Math (per core): with d=0.99, D=d^128, block index n, in-block offsets
s,t (keys/queries):
  out[t] = sum_{s<=t} d^(t_g - s_g) * q_t k_s * v_s  @ Wo      (t_g global)
All decay factors are folded into host-precomputed constants so the device
only does matmuls + one elementwise mask multiply + a running-sum state:
  - masks[n][s,t]  = 1[s<=t] d^(-s-1) D^-(n-1)        (intra-block, bf16)
  - vscale[n][t]   = (1-d) d^(127-t) D^-n             (v' for state update)
  - escale[n][t]   = d^(t+1) D^(n-1)                  (final ACT evac scale)
  - state S~ = sum_m k_m^T v'_m  (pure running sum, bf16 in SBUF)
The geometric growth of D^-n keeps every intermediate within fp32/bf16
range (max ~1e19) and makes bf16 rounding of the running sum benign
(validated: rel err ~4e-3 vs fp32 reference).
"""

import os
import sys

for _p in (
    "/root/.axon_site",
    "/root/.axon_site/_ro/trn_rl_repo",
    "/root/.axon_site/_ro/pypackages",
    "/opt/trn_rl_repo",
):
    if os.path.isdir(_p) and _p not in sys.path:
        sys.path.append(_p)

import numpy as np
import ml_dtypes
from contextlib import ExitStack

import concourse.bacc as bacc
import concourse.tile as tile
from concourse import mybir
from concourse.bass_utils import run_bass_kernel_spmd

DECAY = 0.99
TBLK = 128
BF16 = ml_dtypes.bfloat16
BF = mybir.dt.bfloat16
F32 = mybir.dt.float32


def build_nc(T=4096, E=1024, Dk=1024, Dvh=512, ST=512, pair_groups=None):
    """Build the per-core Bass program. Same program runs on all 8 cores
    (SPMD); only the input data differs.

    q/k phase-A work is split across the two cores of a pair: each core
    computes sigmoid(x @ Wq_half + b_half) for the dk-half whose weights it
    was GIVEN as input (wq input is [E, Dk/2]), then the halves are
    exchanged with a pairwise AllGather through DRAM bounce buffers. Core
    2b (group rank 0) always carries the low half, so the gathered layout
    is identical on both cores and the program stays SPMD-symmetric."""
    N = T // TBLK       # number of 128-token blocks
    NU = T // ST        # number of super-tiles
    JB = ST // TBLK     # blocks per super-tile
    EC = E // 128       # E chunks (contraction)
    KC = Dk // 128      # Dk chunks
    HC = KC // 2        # dk chunks computed locally (half)
    DC = Dvh // 128     # Dv-half chunks
    Dkh = Dk // 2
    if pair_groups is None:
        pair_groups = [[0, 1], [2, 3], [4, 5], [6, 7]]

    nc = bacc.Bacc(num_devices=8)
    # all inputs host-pre-arranged to [128-partition, ...contiguous] layout so
    # every DMA is 128 descriptors of 4-8KB (max descriptor efficiency)
    xt = nc.dram_tensor("xt", [NU, 128, EC, ST], BF, kind="ExternalInput")
    wq = nc.dram_tensor("wq", [128, EC, Dkh], BF, kind="ExternalInput")
    wk = nc.dram_tensor("wk", [128, EC, Dkh], BF, kind="ExternalInput")
    wv = nc.dram_tensor("wv", [128, EC, Dvh], BF, kind="ExternalInput")
    wo = nc.dram_tensor("wo", [128, DC, E], BF, kind="ExternalInput")
    bq = nc.dram_tensor("bq", [128, HC], F32, kind="ExternalInput")
    bk = nc.dram_tensor("bk", [128, HC], F32, kind="ExternalInput")
    masks = nc.dram_tensor("masks", [128, N, 128], BF, kind="ExternalInput")
    vscale = nc.dram_tensor("vscale", [128, N], F32, kind="ExternalInput")
    escale = nc.dram_tensor("escale", [128, N], F32, kind="ExternalInput")
    ident = nc.dram_tensor("ident", [128, 128], BF, kind="ExternalInput")
    out = nc.dram_tensor("out", [T, E], F32, kind="ExternalOutput")

    SIG = mybir.ActivationFunctionType.Sigmoid
    CPY = mybir.ActivationFunctionType.Copy

    with tile.TileContext(nc) as tc:
        with ExitStack() as ctx:
            consts = ctx.enter_context(tc.tile_pool(name="consts", bufs=1))
            qkh_pool = ctx.enter_context(tc.tile_pool(name="qkh", bufs=2))
            dram = ctx.enter_context(tc.tile_pool(name="dram", bufs=2, space="DRAM"))
            xt_pool = ctx.enter_context(tc.tile_pool(name="xt", bufs=2))
            qk_pool = ctx.enter_context(tc.tile_pool(name="qk", bufs=2))
            v_pool = ctx.enter_context(tc.tile_pool(name="v", bufs=2))
            kn_pool = ctx.enter_context(tc.tile_pool(name="kn", bufs=3))
            ap_pool = ctx.enter_context(tc.tile_pool(name="apool", bufs=3))
            yt_pool = ctx.enter_context(tc.tile_pool(name="yt", bufs=3))
            o_pool = ctx.enter_context(tc.tile_pool(name="opool", bufs=2))
            st_pool = ctx.enter_context(tc.tile_pool(name="state", bufs=1))
            psA = ctx.enter_context(tc.tile_pool(name="psA", bufs=3, space="PSUM"))
            psB = ctx.enter_context(tc.tile_pool(name="psB", bufs=3, space="PSUM"))
            psY = ctx.enter_context(tc.tile_pool(name="psY", bufs=1, space="PSUM"))
            psa = ctx.enter_context(tc.tile_pool(name="psa", bufs=1, space="PSUM"))

            # ---- constants into SBUF ----
            # per-chunk DMAs so the first phase-A matmuls start as soon as
            # their e-chunk lands instead of waiting for the whole weight
            wq_sb = consts.tile([128, EC, Dkh], BF)
            nc.sync.dma_start(out=wq_sb, in_=wq[:, :, :])
            wk_sb = consts.tile([128, EC, Dkh], BF)
            nc.sync.dma_start(out=wk_sb, in_=wk[:, :, :])
            bq_sb = consts.tile([128, HC], F32)
            nc.sync.dma_start(out=bq_sb, in_=bq[:, :])
            bk_sb = consts.tile([128, HC], F32)
            nc.sync.dma_start(out=bk_sb, in_=bk[:, :])
            vs_sb = consts.tile([128, N], F32)
            nc.sync.dma_start(out=vs_sb, in_=vscale[:, :])
            es_sb = consts.tile([128, N], F32)
            nc.sync.dma_start(out=es_sb, in_=escale[:, :])
            id_sb = consts.tile([128, 128], BF)
            nc.sync.dma_start(out=id_sb, in_=ident[:, :])
            wv_sb = consts.tile([128, EC, Dvh], BF)
            nc.sync.dma_start(out=wv_sb, in_=wv[:, :, :])
            mk_sb = consts.tile([128, N, 128], BF)
            nc.sync.dma_start(out=mk_sb, in_=masks[:, :, :])
            wo_sb = consts.tile([128, DC, E], BF)
            nc.sync.dma_start(out=wo_sb, in_=wo[:, :, :])

            # persistent scaled-sum state S~ [dk, dv], one tile per dk-chunk
            # (separate tiles -> per-chunk dependency chains, so the y2 reads
            # of block n+1 pipeline against the state adds of block n)
            S_c = [st_pool.tile([128, Dvh], BF, name=f"S{c}", tag=f"S{c}")
                   for c in range(KC)]

            def emit_phase_a(u):
                # ---- load xT super-tile ----
                xt_u = xt_pool.tile([128, EC, ST], BF, name="xt_u")
                nc.sync.dma_start(out=xt_u, in_=xt[u])

                # ---- phase A: this core computes its dk-half of qT, kT ----
                qTh_u = qkh_pool.tile([128, HC, ST], BF, name="qTh_u", tag="qTh")
                kTh_u = qkh_pool.tile([128, HC, ST], BF, name="kTh_u", tag="kTh")
                for c in range(HC):
                    ps = psA.tile([128, ST], F32, name="psq", tag="psA")
                    for e in range(EC):
                        nc.tensor.matmul(
                            ps, wq_sb[:, e, c * 128:(c + 1) * 128], xt_u[:, e, :],
                            start=(e == 0), stop=(e == EC - 1))
                    nc.scalar.activation(qTh_u[:, c, :], ps, SIG, bias=bq_sb[:, c:c + 1])
                for c in range(HC):
                    ps = psA.tile([128, ST], F32, name="psk", tag="psA")
                    for e in range(EC):
                        nc.tensor.matmul(
                            ps, wk_sb[:, e, c * 128:(c + 1) * 128], xt_u[:, e, :],
                            start=(e == 0), stop=(e == EC - 1))
                    nc.scalar.activation(kTh_u[:, c, :], ps, SIG, bias=bk_sb[:, c:c + 1])

                # ---- exchange halves with the pair partner (AllGather) ----
                bin_u = dram.tile([2, 128, HC, ST], BF, name="bin_u", tag="bin")
                nc.sync.dma_start(out=bin_u[0], in_=qTh_u)
                nc.sync.dma_start(out=bin_u[1], in_=kTh_u)
                bout_u = dram.tile([2, 2, 128, HC, ST], BF, name="bout_u", tag="bout")
                nc.gpsimd.collective_compute(
                    "AllGather", mybir.AluOpType.bypass,
                    replica_groups=pair_groups,
                    ins=[bin_u.opt()], outs=[bout_u.opt()])
                qT_u = qk_pool.tile([128, KC, ST], BF, name="qT_u", tag="qT")
                kT_u = qk_pool.tile([128, KC, ST], BF, name="kT_u", tag="kT")
                for g in range(2):
                    nc.sync.dma_start(
                        out=qT_u[:, g * HC:(g + 1) * HC, :], in_=bout_u[g, 0])
                    nc.sync.dma_start(
                        out=kT_u[:, g * HC:(g + 1) * HC, :], in_=bout_u[g, 1])

                v_u = v_pool.tile([128, JB, Dvh], BF, name="v_u", tag="v")
                vp_u = v_pool.tile([128, JB, Dvh], BF, name="vp_u", tag="vp")
                for j in range(JB):
                    n = u * JB + j
                    ps = psA.tile([128, ST], F32, name="psv", tag="psA")[:, :Dvh]
                    for e in range(EC):
                        nc.tensor.matmul(
                            ps, xt_u[:, e, j * 128:(j + 1) * 128], wv_sb[:, e, :],
                            start=(e == 0), stop=(e == EC - 1))
                    nc.scalar.activation(v_u[:, j, :], ps, CPY, scale=1.0 - DECAY)
                    nc.scalar.activation(vp_u[:, j, :], ps, CPY, scale=vs_sb[:, n:n + 1])
                return qT_u, kT_u, v_u, vp_u

            def emit_blocks(u, tiles):
                qT_u, kT_u, v_u, vp_u = tiles
                # ---- block loop ----
                for j in range(JB):
                    n = u * JB + j
                    jsl = slice(j * 128, (j + 1) * 128)

                    # intra-block attention logits a^T[s,t], masked
                    a_ps = psa.tile([128, 128], F32, name="a_ps")
                    for c in range(KC):
                        nc.tensor.matmul(
                            a_ps, kT_u[:, c, jsl], qT_u[:, c, jsl],
                            start=(c == 0), stop=(c == KC - 1))
                    a_sb = ap_pool.tile([128, 128], BF, name="a_sb")
                    nc.vector.tensor_mul(a_sb, a_ps, mk_sb[:, n, :])

                    # k natural [s, dk] via PE transposes of kT
                    kn = kn_pool.tile([128, Dk], BF, name="kn")
                    for h in range(KC // 4):
                        tp = psB.tile([128, 1024], BF, name="tp", tag="psB")[:, :512]
                        for q4 in range(4):
                            c = h * 4 + q4
                            nc.tensor.transpose(
                                tp[:, q4 * 128:(q4 + 1) * 128],
                                kT_u[:, c, jsl], id_sb)
                        if h == 0:
                            nc.scalar.copy(kn[:, h * 512:(h + 1) * 512], tp)
                        else:
                            nc.vector.tensor_copy(kn[:, h * 512:(h + 1) * 512], tp)

                    # yT[dv, t] = v^T a' + S~^T-contract (cross), unscaled
                    y_ps = psY.tile([128, 4 * 128], F32, name="y_ps")
                    for dc in range(DC):
                        osl = slice(dc * 128, (dc + 1) * 128)
                        dvsl = slice(dc * 128, (dc + 1) * 128)
                        nc.tensor.matmul(
                            y_ps[:, osl], v_u[:, j, dvsl], a_sb,
                            start=True, stop=(n == 0))
                        if n > 0:
                            for c in range(KC):
                                nc.tensor.matmul(
                                    y_ps[:, osl], S_c[c][:, dvsl], qT_u[:, c, jsl],
                                    start=False, stop=(c == KC - 1))
                    yT_sb = yt_pool.tile([128, 4 * 128], BF, name="yT_sb")
                    nc.vector.tensor_copy(yT_sb, y_ps)

                    # state update: S~ += k^T v'  (running sum; copy at n=0).
                    # Emitted before the out projection so the DVE adds overlap
                    # the out matmuls and next block's aT/transposes on PE.
                    for c in range(KC):
                        kv_ps = psB.tile([128, 512], F32, name="kv_ps", tag="psB")[:, :Dvh]
                        nc.tensor.matmul(
                            kv_ps, kn[:, c * 128:(c + 1) * 128], vp_u[:, j, :],
                            start=True, stop=True)
                        if n == 0:
                            nc.vector.tensor_copy(S_c[c], kv_ps)
                        else:
                            nc.vector.tensor_add(S_c[c], S_c[c], kv_ps)

                    # out[t, e] = yT^T @ Wo, evacuated with escale[n][t]
                    o_sb = o_pool.tile([128, E], F32, name="o_sb")
                    for hh in range(E // 512):
                        o_ps = psB.tile([128, 512], F32, name="o_ps", tag="psB")
                        for dc in range(DC):
                            nc.tensor.matmul(
                                o_ps, yT_sb[:, dc * 128:(dc + 1) * 128],
                                wo_sb[:, dc, hh * 512:(hh + 1) * 512],
                                start=(dc == 0), stop=(dc == DC - 1))
                        nc.scalar.activation(
                            o_sb[:, hh * 512:(hh + 1) * 512], o_ps, CPY,
                            scale=es_sb[:, n:n + 1])
                    nc.sync.dma_start(
                        out=out[n * 128:(n + 1) * 128, :], in_=o_sb)

            # Software pipeline: emit phase A one super-tile ahead so the
            # AllGather for u+1 is in flight during the block loop of u.
            pend = emit_phase_a(0)
            for u in range(NU):
                nxt = emit_phase_a(u + 1) if u + 1 < NU else None
                emit_blocks(u, pend)
                pend = nxt
    return nc


def make_host_constants(T=4096, dtype_np=np.float32):
    """Host-precomputed decay constants (see module docstring)."""
    N = T // TBLK
    d = np.float64(DECAY)
    D128 = d ** TBLK
    s = np.arange(TBLK, dtype=np.float64)
    t = np.arange(TBLK, dtype=np.float64)
    nn = np.arange(N, dtype=np.float64)

    # masks[s, n, t] = 1[s<=t] * d^(-s-1) * D128^-(n-1)
    tri = (s[:, None] <= t[None, :]).astype(np.float64)  # [s, t]
    m = tri[:, None, :] * (d ** (-s - 1.0))[:, None, None] \
        * (D128 ** (-(nn - 1.0)))[None, :, None]
    masks = m.astype(BF16)

    # vscale[t, n] = (1-d) d^(127-t) D128^-n
    vsc = ((1.0 - d) * d ** (127.0 - t))[:, None] * (D128 ** (-nn))[None, :]
    vscale = vsc.astype(np.float32)

    # escale[t, n] = d^(t+1) D128^(n-1)
    esc = (d ** (t + 1.0))[:, None] * (D128 ** (nn - 1.0))[None, :]
    escale = esc.astype(np.float32)

    ident = np.eye(128, dtype=BF16)
    return masks, vscale, escale, ident


_NC_CACHE = {}


def _get_nc(T, E, Dk, Dvh):
    key = (T, E, Dk, Dvh)
    if key not in _NC_CACHE:
        nc = build_nc(T=T, E=E, Dk=Dk, Dvh=Dvh)
        nc.finalize()
        _NC_CACHE[key] = nc
    return _NC_CACHE[key]


def kernel(x, Wv, Wk, bk, Wq, bq, Wo):
    y, _ = run(x, Wv, Wk, bk, Wq, bq, Wo)
    return y


def _install_ntff_hook():
    """The agent image's antenv lacks axon_hooks; recreate it from
    trn_boot's ctypes NTFF driver so trace=True produces profiles."""
    try:
        from antenv.axon_hooks import get_axon_ntff_profile_hook  # noqa: F401
        return
    except ImportError:
        pass
    try:
        import types
        import antenv
        from trn_agent_boot.trn_boot import _ntff_profile_via_ctypes
        hook = _ntff_profile_via_ctypes("/opt/axon/libaxon_pjrt.so")
        mod = types.ModuleType("antenv.axon_hooks")
        _h = {"hook": hook}
        mod.get_axon_ntff_profile_hook = lambda: _h["hook"]
        mod.set_axon_ntff_profile_hook = lambda h: _h.update(hook=h)
        sys.modules["antenv.axon_hooks"] = mod
        antenv.axon_hooks = mod
    except Exception as e:  # profiling is best-effort
        print(f"ntff hook install failed: {e}")


def _arrange_xt(xb, ST=512):
    """x[b] [T, E] -> xT pre-tiled [NU, 128, EC, ST] bf16, contiguous."""
    T, E = xb.shape
    xT = np.ascontiguousarray(xb.T).astype(BF16)          # [E, T]
    EC, NU = E // 128, T // ST
    return np.ascontiguousarray(
        xT.reshape(EC, 128, NU, ST).transpose(2, 1, 0, 3))


def _arrange_w(w):
    """[E-or-Dv, D] -> [128, chunks, D] with row = chunk*128 + p."""
    R, D = w.shape
    C = R // 128
    return np.ascontiguousarray(w.reshape(C, 128, D).transpose(1, 0, 2))


def _arrange_b(b):
    b = np.asarray(b, np.float32).reshape(-1)
    C = b.shape[0] // 128
    return np.ascontiguousarray(b.reshape(C, 128).T)


def run(x, Wv, Wk, bk, Wq, bq, Wo, trace=False):
    x = np.asarray(x)
    B, T, E = x.shape
    Dk = np.asarray(Wk).shape[1]
    Dv = np.asarray(Wv).shape[1]
    Dvh = Dv // 2
    assert B == 4, "sharding is hardcoded for B=4 x 2 Dv-halves"

    nc = _get_nc(T, E, Dk, Dvh)
    masks, vscale, escale, ident = make_host_constants(T=T)

    wq_bf = np.asarray(Wq, BF16)
    wk_bf = np.asarray(Wk, BF16)
    bq32 = np.asarray(bq, np.float32).reshape(Dk, 1)
    bk32 = np.asarray(bk, np.float32).reshape(Dk, 1)
    Dkh = Dk // 2

    in_maps = []
    for c in range(8):
        b, h = divmod(c, 2)
        dvs = slice(h * Dvh, (h + 1) * Dvh)
        # this core computes the q/k dk-half matching its pair rank
        dks = slice(h * Dkh, (h + 1) * Dkh)
        in_maps.append({
            "xt": _arrange_xt(x[b]),
            "wq": _arrange_w(wq_bf[:, dks]),
            "wk": _arrange_w(wk_bf[:, dks]),
            "wv": _arrange_w(np.asarray(Wv[:, dvs], BF16)),
            "wo": _arrange_w(np.asarray(Wo[dvs], BF16)),
            "bq": _arrange_b(bq32[dks]),
            "bk": _arrange_b(bk32[dks]),
            "masks": masks,
            "vscale": vscale,
            "escale": escale,
            "ident": ident,
        })

    if trace:
        _install_ntff_hook()
    res = run_bass_kernel_spmd(nc, in_maps, core_ids=list(range(8)), trace=trace)
    y = np.zeros((B, T, E), np.float32)
    for c in range(8):
        b = c // 2
        y[b] += res.results[c]["out"]
    return y, res


# revision 22
# speedup vs baseline: 1.1139x; 1.0305x over previous
"""Trainium2 Bass kernel for nn_Decay2DBlk (block-decay linear attention).

Full-input contract: kernel(**inputs) takes the unsharded inputs from
setup_inputs() and returns the full [B, T, E] output.

Sharding: 8 cores = 4 batch elements x 2 Dv-halves. Each core computes a
partial output y_b_h = (attn(x_b) restricted to its Dv half) @ Wo[half];
the host sums the two partials per batch element (the "all-reduce after
w_out" done host-side since outputs are gathered anyway).

Math (per core): with d=0.99, D=d^128, block index n, in-block offsets
s,t (keys/queries):
  out[t] = sum_{s<=t} d^(t_g - s_g) * q_t k_s * v_s  @ Wo      (t_g global)
All decay factors are folded into host-precomputed constants so the device
only does matmuls + one elementwise mask multiply + a running-sum state:
  - masks[n][s,t]  = 1[s<=t] d^(-s-1) D^-(n-1)        (intra-block, bf16)
  - vscale[n][t]   = (1-d) d^(127-t) D^-n             (v' for state update)
  - escale[n][t]   = d^(t+1) D^(n-1)                  (final ACT evac scale)
  - state S~ = sum_m k_m^T v'_m  (pure running sum, bf16 in SBUF)
The geometric growth of D^-n keeps every intermediate within fp32/bf16
range (max ~1e19) and makes bf16 rounding of the running sum benign
(validated: rel err ~4e-3 vs fp32 reference).
"""

import os
import sys

for _p in (
    "/root/.axon_site",
    "/root/.axon_site/_ro/trn_rl_repo",
    "/root/.axon_site/_ro/pypackages",
    "/opt/trn_rl_repo",
):
    if os.path.isdir(_p) and _p not in sys.path:
        sys.path.append(_p)

import numpy as np
import ml_dtypes
from contextlib import ExitStack

import concourse.bacc as bacc
import concourse.tile as tile
from concourse import mybir
from concourse.bass_utils import run_bass_kernel_spmd

DECAY = 0.99
TBLK = 128
BF16 = ml_dtypes.bfloat16
BF = mybir.dt.bfloat16
F32 = mybir.dt.float32


def build_nc(T=4096, E=1024, Dk=1024, Dvh=512, ST=512, pair_groups=None):
    """Build the per-core Bass program. Same program runs on all 8 cores
    (SPMD); only the input data differs.

    q/k phase-A work is split across the two cores of a pair: each core
    computes sigmoid(x @ Wq_half + b_half) for the dk-half whose weights it
    was GIVEN as input (wq input is [E, Dk/2]), then the halves are
    exchanged with a pairwise AllGather through DRAM bounce buffers. Core
    2b (group rank 0) always carries the low half, so the gathered layout
    is identical on both cores and the program stays SPMD-symmetric."""
    N = T // TBLK       # number of 128-token blocks
    NU = T // ST        # number of super-tiles
    JB = ST // TBLK     # blocks per super-tile
    EC = E // 128       # E chunks (contraction)
    KC = Dk // 128      # Dk chunks
    HC = KC // 2        # dk chunks computed locally (half)
    DC = Dvh // 128     # Dv-half chunks
    Dkh = Dk // 2
    if pair_groups is None:
        pair_groups = [[0, 1], [2, 3], [4, 5], [6, 7]]

    nc = bacc.Bacc(num_devices=8)
    # all inputs host-pre-arranged to [128-partition, ...contiguous] layout so
    # every DMA is 128 descriptors of 4-8KB (max descriptor efficiency)
    xt = nc.dram_tensor("xt", [NU, 128, EC, ST], BF, kind="ExternalInput")
    wq = nc.dram_tensor("wq", [128, EC, Dkh], BF, kind="ExternalInput")
    wk = nc.dram_tensor("wk", [128, EC, Dkh], BF, kind="ExternalInput")
    wv = nc.dram_tensor("wv", [128, EC, Dvh], BF, kind="ExternalInput")
    wo = nc.dram_tensor("wo", [128, DC, E], BF, kind="ExternalInput")
    bq = nc.dram_tensor("bq", [128, HC], F32, kind="ExternalInput")
    bk = nc.dram_tensor("bk", [128, HC], F32, kind="ExternalInput")
    masks = nc.dram_tensor("masks", [128, N, 128], BF, kind="ExternalInput")
    vscale = nc.dram_tensor("vscale", [128, N], F32, kind="ExternalInput")
    escale = nc.dram_tensor("escale", [128, N], F32, kind="ExternalInput")
    ident = nc.dram_tensor("ident", [128, 128], BF, kind="ExternalInput")
    out = nc.dram_tensor("out", [T, E], F32, kind="ExternalOutput")

    SIG = mybir.ActivationFunctionType.Sigmoid
    CPY = mybir.ActivationFunctionType.Copy

    with tile.TileContext(nc) as tc:
        with ExitStack() as ctx:
            consts = ctx.enter_context(tc.tile_pool(name="consts", bufs=1))
            qkh_pool = ctx.enter_context(tc.tile_pool(name="qkh", bufs=2))
            dram = ctx.enter_context(tc.tile_pool(name="dram", bufs=2, space="DRAM"))
            xt_pool = ctx.enter_context(tc.tile_pool(name="xt", bufs=2))
            qk_pool = ctx.enter_context(tc.tile_pool(name="qk", bufs=3))
            v_pool = ctx.enter_context(tc.tile_pool(name="v", bufs=3))
            kn_pool = ctx.enter_context(tc.tile_pool(name="kn", bufs=3))
            ap_pool = ctx.enter_context(tc.tile_pool(name="apool", bufs=3))
            yt_pool = ctx.enter_context(tc.tile_pool(name="yt", bufs=3))
            o_pool = ctx.enter_context(tc.tile_pool(name="opool", bufs=2))
            st_pool = ctx.enter_context(tc.tile_pool(name="state", bufs=1))
            psA = ctx.enter_context(tc.tile_pool(name="psA", bufs=3, space="PSUM"))
            psB = ctx.enter_context(tc.tile_pool(name="psB", bufs=3, space="PSUM"))
            psY = ctx.enter_context(tc.tile_pool(name="psY", bufs=1, space="PSUM"))
            psa = ctx.enter_context(tc.tile_pool(name="psa", bufs=1, space="PSUM"))

            # ---- constants into SBUF ----
            # per-chunk DMAs so the first phase-A matmuls start as soon as
            # their e-chunk lands instead of waiting for the whole weight
            wq_sb = consts.tile([128, EC, Dkh], BF)
            nc.sync.dma_start(out=wq_sb, in_=wq[:, :, :])
            wk_sb = consts.tile([128, EC, Dkh], BF)
            nc.sync.dma_start(out=wk_sb, in_=wk[:, :, :])
            bq_sb = consts.tile([128, HC], F32)
            nc.sync.dma_start(out=bq_sb, in_=bq[:, :])
            bk_sb = consts.tile([128, HC], F32)
            nc.sync.dma_start(out=bk_sb, in_=bk[:, :])
            vs_sb = consts.tile([128, N], F32)
            nc.sync.dma_start(out=vs_sb, in_=vscale[:, :])
            es_sb = consts.tile([128, N], F32)
            nc.sync.dma_start(out=es_sb, in_=escale[:, :])
            id_sb = consts.tile([128, 128], BF)
            nc.sync.dma_start(out=id_sb, in_=ident[:, :])
            wv_sb = consts.tile([128, EC, Dvh], BF)
            nc.sync.dma_start(out=wv_sb, in_=wv[:, :, :])
            mk_sb = consts.tile([128, N, 128], BF)
            nc.sync.dma_start(out=mk_sb, in_=masks[:, :, :])
            wo_sb = consts.tile([128, DC, E], BF)
            nc.sync.dma_start(out=wo_sb, in_=wo[:, :, :])

            # persistent scaled-sum state S~ [dk, dv], one tile per dk-chunk
            # (separate tiles -> per-chunk dependency chains, so the y2 reads
            # of block n+1 pipeline against the state adds of block n)
            S_c = [st_pool.tile([128, Dvh], BF, name=f"S{c}", tag=f"S{c}")
                   for c in range(KC)]

            def emit_phase_a(u):
                # ---- load xT super-tile ----
                xt_u = xt_pool.tile([128, EC, ST], BF, name="xt_u")
                nc.sync.dma_start(out=xt_u, in_=xt[u])

                # ---- phase A: this core computes its dk-half of qT, kT ----
                qTh_u = qkh_pool.tile([128, HC, ST], BF, name="qTh_u", tag="qTh")
                kTh_u = qkh_pool.tile([128, HC, ST], BF, name="kTh_u", tag="kTh")
                for c in range(HC):
                    ps = psA.tile([128, ST], F32, name="psq", tag="psA")
                    for e in range(EC):
                        nc.tensor.matmul(
                            ps, wq_sb[:, e, c * 128:(c + 1) * 128], xt_u[:, e, :],
                            start=(e == 0), stop=(e == EC - 1))
                    nc.scalar.activation(qTh_u[:, c, :], ps, SIG, bias=bq_sb[:, c:c + 1])
                for c in range(HC):
                    ps = psA.tile([128, ST], F32, name="psk", tag="psA")
                    for e in range(EC):
                        nc.tensor.matmul(
                            ps, wk_sb[:, e, c * 128:(c + 1) * 128], xt_u[:, e, :],
                            start=(e == 0), stop=(e == EC - 1))
                    nc.scalar.activation(kTh_u[:, c, :], ps, SIG, bias=bk_sb[:, c:c + 1])

                # ---- exchange halves with the pair partner (AllGather) ----
                bin_u = dram.tile([2, 128, HC, ST], BF, name="bin_u", tag="bin")
                nc.gpsimd.dma_start(out=bin_u[0], in_=qTh_u)
                nc.gpsimd.dma_start(out=bin_u[1], in_=kTh_u)
                bout_u = dram.tile([2, 2, 128, HC, ST], BF, name="bout_u", tag="bout")
                nc.gpsimd.collective_compute(
                    "AllGather", mybir.AluOpType.bypass,
                    replica_groups=pair_groups,
                    ins=[bin_u.opt()], outs=[bout_u.opt()])
                qT_u = qk_pool.tile([128, KC, ST], BF, name="qT_u", tag="qT")
                kT_u = qk_pool.tile([128, KC, ST], BF, name="kT_u", tag="kT")
                for g in range(2):
                    nc.gpsimd.dma_start(
                        out=qT_u[:, g * HC:(g + 1) * HC, :], in_=bout_u[g, 0])
                    nc.gpsimd.dma_start(
                        out=kT_u[:, g * HC:(g + 1) * HC, :], in_=bout_u[g, 1])

                v_u = v_pool.tile([128, JB, Dvh], BF, name="v_u", tag="v")
                vp_u = v_pool.tile([128, JB, Dvh], BF, name="vp_u", tag="vp")
                for j in range(JB):
                    n = u * JB + j
                    ps = psA.tile([128, ST], F32, name="psv", tag="psA")[:, :Dvh]
                    for e in range(EC):
                        nc.tensor.matmul(
                            ps, xt_u[:, e, j * 128:(j + 1) * 128], wv_sb[:, e, :],
                            start=(e == 0), stop=(e == EC - 1))
                    nc.scalar.activation(v_u[:, j, :], ps, CPY, scale=1.0 - DECAY)
                    nc.scalar.activation(vp_u[:, j, :], ps, CPY, scale=vs_sb[:, n:n + 1])
                return qT_u, kT_u, v_u, vp_u

            def emit_blocks(u, tiles):
                qT_u, kT_u, v_u, vp_u = tiles
                # ---- block loop ----
                for j in range(JB):
                    n = u * JB + j
                    jsl = slice(j * 128, (j + 1) * 128)

                    # intra-block attention logits a^T[s,t], masked
                    a_ps = psa.tile([128, 128], F32, name="a_ps")
                    for c in range(KC):
                        nc.tensor.matmul(
                            a_ps, kT_u[:, c, jsl], qT_u[:, c, jsl],
                            start=(c == 0), stop=(c == KC - 1))
                    a_sb = ap_pool.tile([128, 128], BF, name="a_sb")
                    nc.vector.tensor_mul(a_sb, a_ps, mk_sb[:, n, :])

                    # k natural [s, dk] via PE transposes of kT
                    kn = kn_pool.tile([128, Dk], BF, name="kn")
                    for h in range(KC // 4):
                        tp = psB.tile([128, 1024], BF, name="tp", tag="psB")[:, :512]
                        for q4 in range(4):
                            c = h * 4 + q4
                            nc.tensor.transpose(
                                tp[:, q4 * 128:(q4 + 1) * 128],
                                kT_u[:, c, jsl], id_sb)
                        if h == 0:
                            nc.scalar.copy(kn[:, h * 512:(h + 1) * 512], tp)
                        else:
                            nc.vector.tensor_copy(kn[:, h * 512:(h + 1) * 512], tp)

                    # yT[dv, t] = v^T a' + S~^T-contract (cross), unscaled
                    y_ps = psY.tile([128, 4 * 128], F32, name="y_ps")
                    for dc in range(DC):
                        osl = slice(dc * 128, (dc + 1) * 128)
                        dvsl = slice(dc * 128, (dc + 1) * 128)
                        nc.tensor.matmul(
                            y_ps[:, osl], v_u[:, j, dvsl], a_sb,
                            start=True, stop=(n == 0))
                        if n > 0:
                            for c in range(KC):
                                nc.tensor.matmul(
                                    y_ps[:, osl], S_c[c][:, dvsl], qT_u[:, c, jsl],
                                    start=False, stop=(c == KC - 1))
                    yT_sb = yt_pool.tile([128, 4 * 128], BF, name="yT_sb")
                    nc.scalar.copy(yT_sb, y_ps)

                    # state update: S~ += k^T v'  (running sum; copy at n=0).
                    # Emitted before the out projection so the DVE adds overlap
                    # the out matmuls and next block's aT/transposes on PE.
                    for c in range(KC):
                        kv_ps = psB.tile([128, 512], F32, name="kv_ps", tag="psB")[:, :Dvh]
                        nc.tensor.matmul(
                            kv_ps, kn[:, c * 128:(c + 1) * 128], vp_u[:, j, :],
                            start=True, stop=True)
                        if n == 0:
                            nc.vector.tensor_copy(S_c[c], kv_ps)
                        else:
                            nc.vector.tensor_add(S_c[c], S_c[c], kv_ps)

                    # out[t, e] = yT^T @ Wo, evacuated with escale[n][t]
                    o_sb = o_pool.tile([128, E], F32, name="o_sb")
                    for hh in range(E // 512):
                        o_ps = psB.tile([128, 512], F32, name="o_ps", tag="psB")
                        for dc in range(DC):
                            nc.tensor.matmul(
                                o_ps, yT_sb[:, dc * 128:(dc + 1) * 128],
                                wo_sb[:, dc, hh * 512:(hh + 1) * 512],
                                start=(dc == 0), stop=(dc == DC - 1))
                        nc.scalar.activation(
                            o_sb[:, hh * 512:(hh + 1) * 512], o_ps, CPY,
                            scale=es_sb[:, n:n + 1])
                    nc.sync.dma_start(
                        out=out[n * 128:(n + 1) * 128, :], in_=o_sb)

            # Software pipeline, depth 2: phase A (and its AllGather) for
            # u+2 is issued before the block loop of u, so the exchange has
            # two whole block-loops of slack to complete.
            pend = {0: emit_phase_a(0)}
            if NU > 1:
                pend[1] = emit_phase_a(1)
            for u in range(NU):
                if u + 2 < NU:
                    pend[u + 2] = emit_phase_a(u + 2)
                emit_blocks(u, pend.pop(u))
    return nc


def make_host_constants(T=4096, dtype_np=np.float32):
    """Host-precomputed decay constants (see module docstring)."""
    N = T // TBLK
    d = np.float64(DECAY)
    D128 = d ** TBLK
    s = np.arange(TBLK, dtype=np.float64)
    t = np.arange(TBLK, dtype=np.float64)
    nn = np.arange(N, dtype=np.float64)

    # masks[s, n, t] = 1[s<=t] * d^(-s-1) * D128^-(n-1)
    tri = (s[:, None] <= t[None, :]).astype(np.float64)  # [s, t]
    m = tri[:, None, :] * (d ** (-s - 1.0))[:, None, None] \
        * (D128 ** (-(nn - 1.0)))[None, :, None]
    masks = m.astype(BF16)

    # vscale[t, n] = (1-d) d^(127-t) D128^-n
    vsc = ((1.0 - d) * d ** (127.0 - t))[:, None] * (D128 ** (-nn))[None, :]
    vscale = vsc.astype(np.float32)

    # escale[t, n] = d^(t+1) D128^(n-1)
    esc = (d ** (t + 1.0))[:, None] * (D128 ** (nn - 1.0))[None, :]
    escale = esc.astype(np.float32)

    ident = np.eye(128, dtype=BF16)
    return masks, vscale, escale, ident


_NC_CACHE = {}


def _get_nc(T, E, Dk, Dvh):
    key = (T, E, Dk, Dvh)
    if key not in _NC_CACHE:
        nc = build_nc(T=T, E=E, Dk=Dk, Dvh=Dvh)
        nc.finalize()
        _NC_CACHE[key] = nc
    return _NC_CACHE[key]


def kernel(x, Wv, Wk, bk, Wq, bq, Wo):
    y, _ = run(x, Wv, Wk, bk, Wq, bq, Wo)
    return y


def _install_ntff_hook():
    """The agent image's antenv lacks axon_hooks; recreate it from
    trn_boot's ctypes NTFF driver so trace=True produces profiles."""
    try:
        from antenv.axon_hooks import get_axon_ntff_profile_hook  # noqa: F401
        return
    except ImportError:
        pass
    try:
        import types
        import antenv
        from trn_agent_boot.trn_boot import _ntff_profile_via_ctypes
        hook = _ntff_profile_via_ctypes("/opt/axon/libaxon_pjrt.so")
        mod = types.ModuleType("antenv.axon_hooks")
        _h = {"hook": hook}
        mod.get_axon_ntff_profile_hook = lambda: _h["hook"]
        mod.set_axon_ntff_profile_hook = lambda h: _h.update(hook=h)
        sys.modules["antenv.axon_hooks"] = mod
        antenv.axon_hooks = mod
    except Exception as e:  # profiling is best-effort
        print(f"ntff hook install failed: {e}")


def _arrange_xt(xb, ST=512):
    """x[b] [T, E] -> xT pre-tiled [NU, 128, EC, ST] bf16, contiguous."""
    T, E = xb.shape
    xT = np.ascontiguousarray(xb.T).astype(BF16)          # [E, T]
    EC, NU = E // 128, T // ST
    return np.ascontiguousarray(
        xT.reshape(EC, 128, NU, ST).transpose(2, 1, 0, 3))


def _arrange_w(w):
    """[E-or-Dv, D] -> [128, chunks, D] with row = chunk*128 + p."""
    R, D = w.shape
    C = R // 128
    return np.ascontiguousarray(w.reshape(C, 128, D).transpose(1, 0, 2))


def _arrange_b(b):
    b = np.asarray(b, np.float32).reshape(-1)
    C = b.shape[0] // 128
    return np.ascontiguousarray(b.reshape(C, 128).T)


def run(x, Wv, Wk, bk, Wq, bq, Wo, trace=False):
    x = np.asarray(x)
    B, T, E = x.shape
    Dk = np.asarray(Wk).shape[1]
    Dv = np.asarray(Wv).shape[1]
    Dvh = Dv // 2
    assert B == 4, "sharding is hardcoded for B=4 x 2 Dv-halves"

    nc = _get_nc(T, E, Dk, Dvh)
    masks, vscale, escale, ident = make_host_constants(T=T)

    wq_bf = np.asarray(Wq, BF16)
    wk_bf = np.asarray(Wk, BF16)
    bq32 = np.asarray(bq, np.float32).reshape(Dk, 1)
    bk32 = np.asarray(bk, np.float32).reshape(Dk, 1)
    Dkh = Dk // 2

    in_maps = []
    for c in range(8):
        b, h = divmod(c, 2)
        dvs = slice(h * Dvh, (h + 1) * Dvh)
        # this core computes the q/k dk-half matching its pair rank
        dks = slice(h * Dkh, (h + 1) * Dkh)
        in_maps.append({
            "xt": _arrange_xt(x[b]),
            "wq": _arrange_w(wq_bf[:, dks]),
            "wk": _arrange_w(wk_bf[:, dks]),
            "wv": _arrange_w(np.asarray(Wv[:, dvs], BF16)),
            "wo": _arrange_w(np.asarray(Wo[dvs], BF16)),
            "bq": _arrange_b(bq32[dks]),
            "bk": _arrange_b(bk32[dks]),
            "masks": masks,
            "vscale": vscale,
            "escale": escale,
            "ident": ident,
        })

    if trace:
        _install_ntff_hook()
    res = run_bass_kernel_spmd(nc, in_maps, core_ids=list(range(8)), trace=trace)
    y = np.zeros((B, T, E), np.float32)
    for c in range(8):
        b = c // 2
        y[b] += res.results[c]["out"]
    return y, res
